# revision 11
# baseline (speedup 1.0000x reference)
"""Trainium2 Bass kernel for DigitCapsules dynamic routing.

Problem: u [256, 2048, 8] f32, W [1, 2048, 10, 16, 8] f32
  u_hat = einsum('pcoi,bpi->bpco', W[0], u)
  3 routing iterations (softmax over c, weighted sum over p, squash,
  agreement update) -> v [256, 10, 16] f32.

Strategy (8 cores, data-parallel over batch, 32 batch elems per core):
  - Partition layout: slabs of 16 p-values; SBUF partition index =
    (p_local * 8 + b_octet_member); contraction on the PE array runs over
    K = (p_local 16, i 8) = 128 using a block-diagonal stationary operand
    u_bd (built on host, zeros are free at matmul time).
  - u_hat materialized once per batch-group in SBUF as bf16, layout
    [part=(p,b), free=(slab, o, c)] -- c innermost so that both big DVE
    ops hit the 2x bf16 perf mode, and the PSUM evacuation performs the
    (c,o)->(o,c) transpose for free.
  - Routing iteration 1 needs no u_hat read: s1 = 0.1 * sum_p u_hat comes
    from a dense-u stationary matmul accumulated over all slabs.
  - Iterations 2,3: G (agreement) via DVE mul + add-tree over o;
    softmax over c via ACT exp + DVE segmented reduce; weighted s-sum
    via PE matmuls with a constant block-diagonal ones stationary.
"""

import numpy as np
import ml_dtypes

bf16 = ml_dtypes.bfloat16

# Problem constants (fixed by the problem spec; do not read spec.json here)
B, P, C, O, IN = 256, 2048, 10, 16, 8
NCORES = 8
B_LOC = B // NCORES          # 32 batch elems per core
NOCT = 2                     # octets per group
BT = NOCT * 8                # 16 batch elems per group
NGROUP = B_LOC // BT         # 2 groups per core
PSLAB = 16                   # p-values per slab
NSLAB = P // PSLAB           # 128 slabs
CO = C * O                   # 160
ROUTING_ITERS = 3
EPS = 1e-9


def _host_prep(u_core, W0, nslab=NSLAB, ngroup=NGROUP):
    """Build the host-side reordered arrays for one core.

    u_core: [ngroup*BT, nslab*PSLAB, IN] f32, W0: [nslab*PSLAB, C, O, IN].
    Returns dict of numpy arrays keyed by DRAM parameter name.
    """
    # W slabs: w_slabs[s, p*8+i, c*16+o] = W0[16s+p, c, o, i]
    w = W0.reshape(nslab, PSLAB, C, O, IN)
    w_slabs = np.ascontiguousarray(
        w.transpose(0, 1, 4, 2, 3).reshape(nslab, PSLAB * IN, CO)
    ).astype(bf16)

    # x[g, t, b, s, p, i] = u_core[g*16 + t*8 + b, 16s+p, i]
    x = u_core.reshape(ngroup, NOCT, 8, nslab, PSLAB, IN)

    # u_bd[g, t, s, p*8+i, p'*8+b] = x[g,t,b,s,p,i] * (p == p')
    xt = np.ascontiguousarray(x.transpose(0, 1, 3, 4, 5, 2))  # [g,t,s,p,i,b]
    u_bd = np.zeros((ngroup, NOCT, nslab, PSLAB, IN, PSLAB, 8), dtype=bf16)
    for p in range(PSLAB):
        u_bd[:, :, :, p, :, p, :] = xt[:, :, :, p]
    u_bd = u_bd.reshape(ngroup, NOCT, nslab, PSLAB * IN, PSLAB * 8)

    # u_t[g, s, p*8+i, t*8+b] = x[g,t,b,s,p,i]  (dense, for the s1 matmul)
    u_t = np.ascontiguousarray(
        x.transpose(0, 3, 4, 5, 1, 2).reshape(ngroup, nslab, PSLAB * IN, BT)
    ).astype(bf16)

    # ones_bd[t, p*8+b, t*8+b] = 1 -- constant stationary for s-reduction;
    # octet t's matmuls write rows t*8..t*8+8 of the [BT, CO] accumulator
    ones_bd = np.zeros((NOCT, PSLAB * 8, BT), dtype=bf16)
    for t in range(NOCT):
        for p in range(PSLAB):
            for b in range(8):
                ones_bd[t, p * 8 + b, t * 8 + b] = 1.0
    return {
        "w_slabs": w_slabs,
        "u_bd": u_bd,
        "u_t": u_t,
        "ones_bd": ones_bd,
    }


def build(nc, tc, ctx, nslab=NSLAB, ngroup=NGROUP):
    """Emit the kernel IR. Parameterized slab/group counts for small tests."""
    import concourse.bass as bass
    from concourse import mybir

    f32 = mybir.dt.float32
    bf = mybir.dt.bfloat16
    Alu = mybir.AluOpType
    Act = mybir.ActivationFunctionType
    Ax = mybir.AxisListType

    b_loc = ngroup * BT

    # ---- DRAM parameters ----
    w_dram = nc.dram_tensor(
        "w_slabs", [nslab, PSLAB * IN, CO], bf, kind="ExternalInput").ap()
    ubd_dram = nc.dram_tensor(
        "u_bd", [ngroup, NOCT, nslab, PSLAB * IN, PSLAB * 8], bf,
        kind="ExternalInput").ap()
    ut_dram = nc.dram_tensor(
        "u_t", [ngroup, nslab, PSLAB * IN, BT], bf, kind="ExternalInput").ap()
    ones_dram = nc.dram_tensor(
        "ones_bd", [NOCT, PSLAB * 8, BT], bf, kind="ExternalInput").ap()
    vout_dram = nc.dram_tensor(
        "v_out", [b_loc, CO], f32, kind="ExternalOutput").ap()
    # internal scratch for partition-replication of v
    vscr_dram = nc.dram_tensor("v_scratch", [BT, CO], bf).ap()

    # ---- pools ----
    consts = ctx.enter_context(tc.tile_pool(name="consts", bufs=1))
    wpool = ctx.enter_context(tc.tile_pool(name="wpool", bufs=3))
    ubdpool = ctx.enter_context(tc.tile_pool(name="ubdpool", bufs=3))
    utpool = ctx.enter_context(tc.tile_pool(name="utpool", bufs=3))
    uhatpool = ctx.enter_context(tc.tile_pool(name="uhat", bufs=1))
    psum = ctx.enter_context(tc.tile_pool(name="psum", bufs=4, space="PSUM"))
    psum_acc = ctx.enter_context(
        tc.tile_pool(name="psum_acc", bufs=1, space="PSUM"))
    small = ctx.enter_context(tc.tile_pool(name="small", bufs=2))
    state = ctx.enter_context(tc.tile_pool(name="state", bufs=1))
    tmp = ctx.enter_context(tc.tile_pool(name="tmp", bufs=2))

    ones_sb = consts.tile([PSLAB * 8, NOCT, BT], bf)
    nc.sync.dma_start(out=ones_sb[:], in_=ones_dram.rearrange("t p n -> p t n"))

    def bcast_ap(ap, insert_pos, size):
        """Insert a stride-0 dim of `size` at free-dim position insert_pos."""
        new = list(ap.ap)
        new.insert(insert_pos, [0, size])
        return bass.AP(tensor=ap.tensor, offset=ap.offset, ap=new)

    def squash_and_store(s_sb, g, it, V_rep):
        """s_sb: [BT, CO] f32 (layout (c,o)). Computes v = squash(s).
        If it < ROUTING_ITERS-1: writes V_rep tiles (bf16, [128, O, C]).
        Else: DMAs v (f32) to v_out rows g*BT..(g+1)*BT."""
        s3 = s_sb[:].rearrange("n (c o) -> n c o", c=C)
        sq = small.tile([BT, CO], f32, tag="sqsq")
        nc.vector.tensor_mul(sq[:].rearrange("n (c o) -> n c o", c=C), s3, s3)
        nrm = small.tile([BT, C], f32, tag="nrm")
        nc.vector.tensor_reduce(
            out=nrm[:], in_=sq[:].rearrange("n (c o) -> n c o", c=C),
            axis=Ax.X, op=Alu.add)
        d1 = small.tile([BT, C], f32, tag="d1")
        nc.vector.tensor_scalar_add(d1[:], nrm[:], 1.0)
        r1 = small.tile([BT, C], f32, tag="r1")
        nc.vector.reciprocal(r1[:], d1[:])
        se = small.tile([BT, C], f32, tag="se")
        nc.vector.tensor_scalar_add(se[:], nrm[:], EPS)
        st = small.tile([BT, C], f32, tag="st")
        nc.scalar.activation(st[:], se[:], Act.Sqrt)
        r2 = small.tile([BT, C], f32, tag="r2")
        nc.vector.reciprocal(r2[:], st[:])
        f1 = small.tile([BT, C], f32, tag="f1")
        nc.vector.tensor_mul(f1[:], nrm[:], r1[:])
        fac = small.tile([BT, C], f32, tag="fac")
        nc.vector.tensor_mul(fac[:], f1[:], r2[:])
        v_sb = small.tile([BT, CO], f32, tag="v_sb")
        nc.vector.tensor_tensor(
            out=v_sb[:].rearrange("n (c o) -> n c o", c=C),
            in0=s3, in1=bcast_ap(fac[:], 2, O), op=Alu.mult)
        if it == ROUTING_ITERS - 1:
            nc.sync.dma_start(
                out=vout_dram[g * BT:(g + 1) * BT, :], in_=v_sb[:])
            return
        # v_bf stored (o,c)-major so the V_rep broadcast DMA is 3-dim
        v_bf = small.tile([BT, O * C], bf, tag="v_bf")
        nc.vector.tensor_copy(
            v_bf[:].rearrange("n (o c) -> n c o", o=O),
            v_sb[:].rearrange("n (c o) -> n c o", c=C))
        nc.sync.dma_start(out=vscr_dram, in_=v_bf[:])
        for t in range(NOCT):
            # V_rep[t][(p,b), o, c] = v[t*8+b, c*16+o], replicated over p
            src = bass.AP(
                tensor=vscr_dram.tensor,
                offset=vscr_dram.offset + t * 8 * CO,
                ap=[[0, PSLAB], [CO, 8], [1, O * C]])
            nc.sync.dma_start(out=V_rep[t][:], in_=src)

    for g in range(ngroup):
        # ---------- Phase A: u_hat materialization + s1 ----------
        uhat = []
        for t in range(NOCT):
            uhat.append(uhatpool.tile([128, nslab, O, C], bf, tag=f"uhat{t}", name=f"uhat{t}"))
        s1_ps = psum_acc.tile([BT, CO], f32, tag="s1ps")
        for s in range(nslab):
            wt = wpool.tile([128, CO], bf)
            nc.sync.dma_start(out=wt[:], in_=w_dram[s])
            ut_t = utpool.tile([128, BT], bf)
            nc.sync.dma_start(out=ut_t[:], in_=ut_dram[g, s])
            nc.tensor.matmul(
                out=s1_ps[:], lhsT=ut_t[:], rhs=wt[:],
                start=(s == 0), stop=(s == nslab - 1))
            for t in range(NOCT):
                ubd_t = ubdpool.tile([128, PSLAB * 8], bf)
                nc.sync.dma_start(out=ubd_t[:], in_=ubd_dram[g, t, s])
                ps = psum.tile([128, CO], f32, tag="uhps")
                nc.tensor.matmul(
                    out=ps[:], lhsT=ubd_t[:], rhs=wt[:],
                    start=True, stop=True)
                # evacuate with (c,o)->(o,c) transpose + bf16 cast
                src = ps[:].rearrange("p (c o) -> p o c", c=C)
                dst = uhat[t][:, s, :, :]
                if t == 0:
                    nc.vector.tensor_copy(dst, src)
                else:
                    nc.scalar.copy(dst, src)

        # ---------- iteration 1: s1 = 0.1 * accumulated ----------
        s_sb = small.tile([BT, CO], f32, tag="s_sb")
        nc.scalar.mul(s_sb[:], s1_ps[:], 1.0 / C)
        V_rep = [state.tile([128, O, C], bf, tag=f"vrep{t}", name=f"vrep{t}", bufs=2)
                 for t in range(NOCT)]
        squash_and_store(s_sb, g, 0, V_rep)

        # ---------- b-state ----------
        bst = [state.tile([128, nslab, C], f32, tag=f"bst{t}", name=f"bst{t}")
               for t in range(NOCT)]
        for t in range(NOCT):
            nc.vector.memset(bst[t][:], 0.0)

        nchunk = max(1, nslab // 32)
        chs = nslab // nchunk  # slabs per chunk

        for it in range(1, ROUTING_ITERS):
            s_ps = psum_acc.tile([BT, CO], f32, tag="sps", name="sps")
            for t in range(NOCT):
                # ---- G-step: b += sum_o uhat * V_rep ----
                for ch in range(nchunk):
                    sl = slice(ch * chs, (ch + 1) * chs)
                    t2 = tmp.tile([128, chs, O, C], bf, tag="t2")
                    nc.vector.tensor_tensor(
                        out=t2[:], in0=uhat[t][:, sl, :, :],
                        in1=bcast_ap(V_rep[t][:], 1, chs), op=Alu.mult)
                    r1 = tmp.tile([128, chs, O // 2, C], bf, tag="r1t")
                    nc.vector.tensor_tensor(
                        out=r1[:], in0=t2[:, :, 0:O // 2, :],
                        in1=t2[:, :, O // 2:O, :], op=Alu.add)
                    r2 = tmp.tile([128, chs, O // 4, C], bf, tag="r2t")
                    nc.vector.tensor_tensor(
                        out=r2[:], in0=r1[:, :, 0:O // 4, :],
                        in1=r1[:, :, O // 4:O // 2, :], op=Alu.add)
                    r3 = tmp.tile([128, chs, 2, C], bf, tag="r3t")
                    nc.vector.tensor_tensor(
                        out=r3[:], in0=r2[:, :, 0:2, :],
                        in1=r2[:, :, 2:4, :], op=Alu.add)
                    r4 = tmp.tile([128, chs, C], bf, tag="r4t")
                    nc.vector.tensor_tensor(
                        out=r4[:], in0=r3[:, :, 0, :], in1=r3[:, :, 1, :],
                        op=Alu.add)
                    nc.vector.tensor_tensor(
                        out=bst[t][:, sl, :], in0=bst[t][:, sl, :],
                        in1=r4[:], op=Alu.add)
                # ---- softmax over c ----
                expt = tmp.tile([128, nslab, C], bf, tag="expt")
                nc.scalar.activation(expt[:], bst[t][:], Act.Exp)
                Z = tmp.tile([128, nslab], f32, tag="Z")
                nc.vector.tensor_reduce(
                    out=Z[:], in_=expt[:], axis=Ax.X, op=Alu.add)
                rz = tmp.tile([128, nslab], f32, tag="rz")
                nc.vector.reciprocal(rz[:], Z[:])
                cw = tmp.tile([128, nslab, C], bf, tag="cw")
                nc.vector.tensor_tensor(
                    out=cw[:], in0=expt[:], in1=bcast_ap(rz[:], 2, C),
                    op=Alu.mult)
                # ---- s-step: premul + PE block-diag ones reduction ----
                for ch in range(nchunk):
                    sl = slice(ch * chs, (ch + 1) * chs)
                    t1 = tmp.tile([128, chs, O, C], bf, tag="t2")
                    nc.vector.tensor_tensor(
                        out=t1[:], in0=uhat[t][:, sl, :, :],
                        in1=bcast_ap(cw[:, sl, :], 2, O), op=Alu.mult)
                    for k in range(chs):
                        s_idx = ch * chs + k
                        nc.tensor.matmul(
                            out=s_ps[:], lhsT=ones_sb[:, t, :],
                            rhs=t1[:, k, :, :],
                            start=(t == 0 and s_idx == 0),
                            stop=(t == NOCT - 1 and s_idx == nslab - 1))
            # collect s (both octets accumulated into one PSUM tile).
            # s_ps free order is (o,c) -- t1's rhs order -- transpose to (c,o)
            s_sb = small.tile([BT, CO], f32, tag="s_sb")
            nc.scalar.copy(s_sb[:].rearrange("n (c o) -> n c o", c=C),
                           s_ps[:].rearrange("n (o c) -> n c o", o=O))
            V_rep_next = [state.tile([128, O, C], bf, tag=f"vrep{t}", name=f"vrepn{t}", bufs=2)
                          for t in range(NOCT)]
            squash_and_store(s_sb, g, it, V_rep_next)
            V_rep = V_rep_next


def make_inputs_per_core(u, W):
    """Full inputs -> list of 8 in_maps."""
    W0 = np.asarray(W, dtype=np.float32)[0]
    u = np.asarray(u, dtype=np.float32)
    in_maps = []
    for c in range(NCORES):
        u_core = u[c * B_LOC:(c + 1) * B_LOC]
        in_maps.append(_host_prep(u_core, W0))
    return in_maps


def numpy_model(u_core, W0):
    """f32 numpy model of the routing (for small-scale checks)."""
    u_hat = np.einsum('pcoi,bpi->bpco', W0, u_core)
    Bl = u_hat.shape[0]
    b = np.zeros(u_hat.shape[:3], dtype=np.float32)
    v = None
    for _ in range(ROUTING_ITERS):
        e = np.exp(b - b.max(axis=2, keepdims=True))
        c = e / e.sum(axis=2, keepdims=True)
        s = np.einsum('bpc,bpco->bco', c, u_hat)
        sq = (s * s).sum(-1, keepdims=True)
        v = (sq / (1 + sq)) * s / np.sqrt(sq + EPS)
        b = b + np.einsum('bpco,bco->bpc', u_hat, v)
    return v


_COMPILED = {}


def _get_compiled():
    if "nc" in _COMPILED:
        return _COMPILED["nc"]
    from contextlib import ExitStack
    import concourse.tile as tile
    from concourse import bacc

    nc = bacc.Bacc("TRN2", target_bir_lowering=False, debug=False,
                   num_devices=NCORES)
    with tile.TileContext(nc) as tc:
        with ExitStack() as ctx:
            build(nc, tc, ctx)
    nc.compile()
    _COMPILED["nc"] = nc
    return nc


def kernel(u, W):
    """Full-input entry point: u [256,2048,8] f32, W [1,2048,10,16,8] f32
    -> v [256, 10, 16] f32."""
    from concourse.bass_utils import run_bass_kernel_spmd

    nc = _get_compiled()
    in_maps = make_inputs_per_core(u, W)
    res = run_bass_kernel_spmd(nc, in_maps, core_ids=list(range(NCORES)))
    outs = [res.results[c]["v_out"] for c in range(NCORES)]
    v = np.concatenate(outs, axis=0).reshape(B, C, O).astype(np.float32)
    return v


# revision 21
# speedup vs baseline: 2.0936x; 2.0936x over previous
"""Trainium2 Bass kernel for DigitCapsules dynamic routing.

Problem: u [256, 2048, 8] f32, W [1, 2048, 10, 16, 8] f32
  u_hat = einsum('pcoi,bpi->bpco', W[0], u)
  3 routing iterations (softmax over c, weighted sum over p, squash,
  agreement update) -> v [256, 10, 16] f32.

Strategy (8 cores, data-parallel over batch, 32 batch elems per core):
  - Partition layout: slabs of 16 p-values; SBUF partition index =
    (p_local * 8 + b_octet_member); contraction on the PE array runs over
    K = (p_local 16, i 8) = 128 using a block-diagonal stationary operand
    u_bd (built on host; its zeros cost nothing at matmul time).
  - u_hat materialized once per batch-group in SBUF as bf16, layout
    [part=(p,b), free=(slab, o, c)] -- c innermost so both big DVE ops
    hit the 2x bf16 perf mode; the PSUM evacuation performs the
    (c,o)->(o,c) transpose for free.
  - Routing iteration 1 needs no u_hat read: s1 = 0.1 * sum_p u_hat from
    a dense-u stationary matmul accumulated over all slabs.
  - Iterations 2,3: G (agreement) via DVE mul + add-tree over o (tree
    tail offloaded to GpSimd); softmax over c via ACT exp + DVE reduce;
    weighted s-sum via PE matmuls with a block-diagonal ones stationary.
  - All host-side arrays are k-major so every DMA is fully contiguous;
    DMAs are spread across engine queues (SP/POOL/ACT/DVE).
"""

import numpy as np
import ml_dtypes

bf16 = ml_dtypes.bfloat16

# Problem constants (fixed by the problem spec; do not read spec.json here)
B, P, C, O, IN = 256, 2048, 10, 16, 8
NCORES = 8
B_LOC = B // NCORES          # 32 batch elems per core
NOCT = 2                     # octets per group
BT = NOCT * 8                # 16 batch elems per group
NGROUP = B_LOC // BT         # 2 groups per core
PSLAB = 16                   # p-values per slab
NSLAB = P // PSLAB           # 128 slabs
CO = C * O                   # 160
ROUTING_ITERS = 3
EPS = 1e-9

CHB = 16    # slabs per u_bd DMA chunk
EV = 2      # slabs per PSUM evacuation batch (1 bank per tile)
SMM = 2     # slabs per s-step matmul (N = SMM*CO = 320 <= 512)


def _host_prep(u_core, W0, nslab=NSLAB, ngroup=NGROUP):
    """Build host-side reordered (k-major, contiguous-DMA) arrays."""
    # w_k[p*8+i, s, c*16+o] = W0[16s+p, c, o, i]
    w = W0.reshape(nslab, PSLAB, C, O, IN)
    w_k = np.ascontiguousarray(
        w.transpose(1, 4, 0, 2, 3).reshape(PSLAB * IN, nslab, CO)
    ).astype(bf16)

    # x[g, t, b, s, p, i] = u_core[g*16 + t*8 + b, 16s+p, i]
    x = u_core.reshape(ngroup, NOCT, 8, nslab, PSLAB, IN)

    # ubd_k[g, t, p*8+i, s, p'*8+b] = x[g,t,b,s,p,i] * (p == p')
    xt = x.transpose(0, 1, 4, 5, 3, 2)  # [g, t, p, i, s, b]
    ubd_k = np.zeros((ngroup, NOCT, PSLAB, IN, nslab, PSLAB, 8), dtype=bf16)
    for p in range(PSLAB):
        ubd_k[:, :, p, :, :, p, :] = xt[:, :, p]
    ubd_k = ubd_k.reshape(ngroup, NOCT, PSLAB * IN, nslab, PSLAB * 8)

    # ut_k[g, p*8+i, s, t*8+b] = x[g,t,b,s,p,i]  (dense, for the s1 matmul)
    ut_k = np.ascontiguousarray(
        x.transpose(0, 4, 5, 3, 1, 2).reshape(ngroup, PSLAB * IN, nslab, BT)
    ).astype(bf16)

    # ones_bd[t, p*8+b, t*8+b] = 1 -- stationary for the s-reduction;
    # octet t's matmuls write rows t*8..t*8+8 of the [BT, N] accumulator
    ones_bd = np.zeros((NOCT, PSLAB * 8, BT), dtype=bf16)
    for t in range(NOCT):
        for p in range(PSLAB):
            for b in range(8):
                ones_bd[t, p * 8 + b, t * 8 + b] = 1.0
    return {
        "w_k": w_k,
        "ubd_k": ubd_k,
        "ut_k": ut_k,
        "ones_bd": ones_bd,
    }


def build(nc, tc, ctx, nslab=NSLAB, ngroup=NGROUP):
    """Emit the kernel IR. Parameterized slab/group counts for small tests."""
    import concourse.bass as bass
    from concourse import mybir

    f32 = mybir.dt.float32
    bf = mybir.dt.bfloat16
    Alu = mybir.AluOpType
    Act = mybir.ActivationFunctionType
    Ax = mybir.AxisListType

    b_loc = ngroup * BT
    chb = min(CHB, nslab)
    ev = min(EV, nslab)
    smm = min(SMM, nslab)

    # ---- DRAM parameters ----
    w_dram = nc.dram_tensor(
        "w_k", [PSLAB * IN, nslab, CO], bf, kind="ExternalInput").ap()
    ubd_dram = nc.dram_tensor(
        "ubd_k", [ngroup, NOCT, PSLAB * IN, nslab, PSLAB * 8], bf,
        kind="ExternalInput").ap()
    ut_dram = nc.dram_tensor(
        "ut_k", [ngroup, PSLAB * IN, nslab, BT], bf,
        kind="ExternalInput").ap()
    ones_dram = nc.dram_tensor(
        "ones_bd", [NOCT, PSLAB * 8, BT], bf, kind="ExternalInput").ap()
    vout_dram = nc.dram_tensor(
        "v_out", [b_loc, CO], f32, kind="ExternalOutput").ap()
    vscr_dram = nc.dram_tensor("v_scratch", [BT, CO], bf).ap()

    # ---- pools ----
    consts = ctx.enter_context(tc.tile_pool(name="consts", bufs=1))
    ubdpool = ctx.enter_context(tc.tile_pool(name="ubdpool", bufs=2))
    utpool = ctx.enter_context(tc.tile_pool(name="utpool", bufs=2))
    uhatpool = ctx.enter_context(tc.tile_pool(name="uhat", bufs=1))
    psum = ctx.enter_context(tc.tile_pool(name="psum", bufs=2, space="PSUM"))
    psum_acc = ctx.enter_context(
        tc.tile_pool(name="psum_acc", bufs=1, space="PSUM"))
    small = ctx.enter_context(tc.tile_pool(name="small", bufs=2))
    state = ctx.enter_context(tc.tile_pool(name="state", bufs=1))
    tmp = ctx.enter_context(tc.tile_pool(name="tmp", bufs=2))

    ones_sb = consts.tile([PSLAB * 8, NOCT, BT], bf)
    nc.sync.dma_start(
        out=ones_sb[:], in_=ones_dram.rearrange("t p n -> p t n"))

    # resident W: whole tensor, loaded in two contiguous halves (ACT queue)
    wall = consts.tile([PSLAB * IN, nslab, CO], bf)
    h = max(1, nslab // 2)
    for j in range(0, nslab, h):
        nc.scalar.dma_start(
            out=wall[:, j:j + h, :], in_=w_dram[:, j:j + h, :])

    def bcast_ap(ap, insert_pos, size):
        """Insert a stride-0 dim of `size` at free-dim position insert_pos."""
        new = list(ap.ap)
        new.insert(insert_pos, [0, size])
        return bass.AP(tensor=ap.tensor, offset=ap.offset, ap=new)

    def squash_and_store(s_sb, g, it, V_rep):
        """s_sb: [BT, CO] f32 (layout (c,o)). v = squash(s).
        it < last: fills V_rep tiles (bf16, [128, O, C]); else DMAs v out."""
        s3 = s_sb[:].rearrange("n (c o) -> n c o", c=C)
        sq = small.tile([BT, CO], f32, tag="sqsq")
        nc.vector.tensor_mul(sq[:].rearrange("n (c o) -> n c o", c=C), s3, s3)
        nrm = small.tile([BT, C], f32, tag="nrm")
        nc.vector.tensor_reduce(
            out=nrm[:], in_=sq[:].rearrange("n (c o) -> n c o", c=C),
            axis=Ax.X, op=Alu.add)
        d1 = small.tile([BT, C], f32, tag="d1")
        nc.vector.tensor_scalar_add(d1[:], nrm[:], 1.0)
        r1 = small.tile([BT, C], f32, tag="r1")
        nc.vector.reciprocal(r1[:], d1[:])
        se = small.tile([BT, C], f32, tag="se")
        nc.vector.tensor_scalar_add(se[:], nrm[:], EPS)
        st = small.tile([BT, C], f32, tag="st")
        nc.scalar.activation(st[:], se[:], Act.Sqrt)
        r2 = small.tile([BT, C], f32, tag="r2")
        nc.vector.reciprocal(r2[:], st[:])
        f1 = small.tile([BT, C], f32, tag="f1")
        nc.vector.tensor_mul(f1[:], nrm[:], r1[:])
        fac = small.tile([BT, C], f32, tag="fac")
        nc.vector.tensor_mul(fac[:], f1[:], r2[:])
        v_sb = small.tile([BT, CO], f32, tag="v_sb")
        nc.vector.tensor_tensor(
            out=v_sb[:].rearrange("n (c o) -> n c o", c=C),
            in0=s3, in1=bcast_ap(fac[:], 2, O), op=Alu.mult)
        if it == ROUTING_ITERS - 1:
            nc.sync.dma_start(
                out=vout_dram[g * BT:(g + 1) * BT, :], in_=v_sb[:])
            return
        # v_bf stored (o,c)-major so the V_rep broadcast DMA is 3-dim
        v_bf = small.tile([BT, O * C], bf, tag="v_bf")
        nc.vector.tensor_copy(
            v_bf[:].rearrange("n (o c) -> n c o", o=O),
            v_sb[:].rearrange("n (c o) -> n c o", c=C))
        nc.sync.dma_start(out=vscr_dram, in_=v_bf[:])
        for t in range(NOCT):
            src = bass.AP(
                tensor=vscr_dram.tensor,
                offset=vscr_dram.offset + t * 8 * CO,
                ap=[[0, PSLAB], [CO, 8], [1, O * C]])
            nc.sync.dma_start(out=V_rep[t][:], in_=src)

    for g in range(ngroup):
        # ---------- Phase A: u_hat materialization + s1 ----------
        uhat = []
        for t in range(NOCT):
            uhat.append(uhatpool.tile(
                [128, nslab, O, C], bf, tag=f"uhat{t}", name=f"uhat{t}"))
        ut_res = utpool.tile([PSLAB * IN, nslab, BT], bf, tag="utres",
                             name="ut_res")
        nc.sync.dma_start(out=ut_res[:], in_=ut_dram[g])
        s1_ps = psum_acc.tile([BT, CO], f32, tag="s1ps")
        for c0 in range(0, nslab, chb):
            ubd_ch = []
            for t in range(NOCT):
                ub = ubdpool.tile([PSLAB * IN, chb, PSLAB * 8], bf,
                                  tag=f"ubd{t}", name=f"ubd{t}")
                eng = nc.sync if t == 0 else nc.gpsimd
                eng.dma_start(out=ub[:], in_=ubd_dram[g, t, :, c0:c0 + chb, :])
                ubd_ch.append(ub)
            for e0 in range(c0, c0 + chb, ev):
                pss = []
                for t in range(NOCT):
                    ps = psum.tile([128, ev, CO], f32, tag=f"ups{t}",
                                   name=f"ups{t}")
                    for q in range(ev):
                        s = e0 + q
                        nc.tensor.matmul(
                            out=ps[:, q, :], lhsT=ubd_ch[t][:, s - c0, :],
                            rhs=wall[:, s, :], start=True, stop=True)
                    pss.append(ps)
                for q in range(ev):
                    s = e0 + q
                    nc.tensor.matmul(
                        out=s1_ps[:], lhsT=ut_res[:, s, :], rhs=wall[:, s, :],
                        start=(s == 0), stop=(s == nslab - 1))
                for t in range(NOCT):
                    src = pss[t][:].rearrange("p s (c o) -> p s o c", c=C)
                    dst = uhat[t][:, e0:e0 + ev, :, :]
                    if t == 0:
                        nc.vector.tensor_copy(dst, src)
                    else:
                        nc.scalar.copy(dst, src)

        # ---------- iteration 1 ----------
        s_sb = small.tile([BT, CO], f32, tag="s_sb")
        nc.scalar.mul(s_sb[:], s1_ps[:], 1.0 / C)
        V_rep = [state.tile([128, O, C], bf, tag=f"vrep{t}",
                            name=f"vrep{t}", bufs=2) for t in range(NOCT)]
        squash_and_store(s_sb, g, 0, V_rep)

        # ---------- b-state (bf16: G magnitudes are <<1) ----------
        bst = [state.tile([128, nslab, C], bf, tag=f"bst{t}",
                          name=f"bst{t}") for t in range(NOCT)]
        for t in range(NOCT):
            nc.vector.memset(bst[t][:], 0.0)

        nchunk = max(1, nslab // 32)
        chs = nslab // nchunk  # slabs per compute chunk

        for it in range(1, ROUTING_ITERS):
            s_ps = psum_acc.tile([BT, smm * CO], f32, tag="sps", name="sps")
            for t in range(NOCT):
                # ---- G-step: bst += sum_o uhat * V_rep ----
                for ch in range(nchunk):
                    sl = slice(ch * chs, (ch + 1) * chs)
                    t2 = tmp.tile([128, chs, O, C], bf, tag="t2")
                    nc.vector.tensor_tensor(
                        out=t2[:], in0=uhat[t][:, sl, :, :],
                        in1=bcast_ap(V_rep[t][:], 1, chs), op=Alu.mult)
                    r1 = tmp.tile([128, chs, O // 2, C], bf, tag="r1t")
                    nc.vector.tensor_tensor(
                        out=r1[:], in0=t2[:, :, 0:O // 2, :],
                        in1=t2[:, :, O // 2:O, :], op=Alu.add)
                    r2 = tmp.tile([128, chs, O // 4, C], bf, tag="r2t", bufs=1)
                    nc.gpsimd.tensor_tensor(
                        out=r2[:], in0=r1[:, :, 0:O // 4, :],
                        in1=r1[:, :, O // 4:O // 2, :], op=Alu.add)
                    r3 = tmp.tile([128, chs, 2, C], bf, tag="r3t", bufs=1)
                    nc.gpsimd.tensor_tensor(
                        out=r3[:], in0=r2[:, :, 0:2, :],
                        in1=r2[:, :, 2:4, :], op=Alu.add)
                    r4 = tmp.tile([128, chs, C], bf, tag="r4t", bufs=1)
                    nc.gpsimd.tensor_tensor(
                        out=r4[:], in0=r3[:, :, 0, :], in1=r3[:, :, 1, :],
                        op=Alu.add)
                    nc.vector.tensor_tensor(
                        out=bst[t][:, sl, :], in0=bst[t][:, sl, :],
                        in1=r4[:], op=Alu.add)
                # ---- softmax over c ----
                expt = tmp.tile([128, nslab, C], bf, tag="expt", bufs=1)
                nc.scalar.activation(expt[:], bst[t][:], Act.Exp)
                Z = tmp.tile([128, nslab], f32, tag="Z")
                nc.vector.tensor_reduce(
                    out=Z[:], in_=expt[:], axis=Ax.X, op=Alu.add)
                rz = tmp.tile([128, nslab], f32, tag="rz")
                nc.vector.reciprocal(rz[:], Z[:])
                cw = tmp.tile([128, nslab, C], bf, tag="cw", bufs=1)
                nc.vector.tensor_tensor(
                    out=cw[:], in0=expt[:], in1=bcast_ap(rz[:], 2, C),
                    op=Alu.mult)
                # ---- s-step: premul + PE block-diag ones reduction ----
                for ch in range(nchunk):
                    sl = slice(ch * chs, (ch + 1) * chs)
                    t1 = tmp.tile([128, chs, O, C], bf, tag="t2")
                    nc.vector.tensor_tensor(
                        out=t1[:], in0=uhat[t][:, sl, :, :],
                        in1=bcast_ap(cw[:, sl, :], 2, O), op=Alu.mult)
                    for k in range(chs // smm):
                        s_idx = ch * chs + k * smm
                        nc.tensor.matmul(
                            out=s_ps[:], lhsT=ones_sb[:, t, :],
                            rhs=t1[:, k * smm:(k + 1) * smm, :, :],
                            start=(t == 0 and s_idx == 0),
                            stop=(t == NOCT - 1 and s_idx == nslab - smm))
            # collect s: sum the smm slab-positions; each is (o,c) ordered
            s_sb = small.tile([BT, CO], f32, tag="s_sb")
            if smm == 2:
                s_rw = small.tile([BT, 2 * CO], f32, tag="s_rw")
                nc.scalar.copy(s_rw[:], s_ps[:])
                nc.vector.tensor_tensor(
                    out=s_sb[:].rearrange("n (c o) -> n c o", c=C),
                    in0=s_rw[:, 0:CO].rearrange("n (o c) -> n c o", o=O),
                    in1=s_rw[:, CO:2 * CO].rearrange("n (o c) -> n c o", o=O),
                    op=Alu.add)
            else:
                nc.scalar.copy(
                    s_sb[:].rearrange("n (c o) -> n c o", c=C),
                    s_ps[:, 0:CO].rearrange("n (o c) -> n c o", o=O))
            V_rep_next = [state.tile([128, O, C], bf, tag=f"vrep{t}",
                                     name=f"vrepn{t}", bufs=2)
                          for t in range(NOCT)]
            squash_and_store(s_sb, g, it, V_rep_next)
            V_rep = V_rep_next


def make_inputs_per_core(u, W):
    """Full inputs -> list of 8 in_maps."""
    W0 = np.asarray(W, dtype=np.float32)[0]
    u = np.asarray(u, dtype=np.float32)
    in_maps = []
    for c in range(NCORES):
        u_core = u[c * B_LOC:(c + 1) * B_LOC]
        in_maps.append(_host_prep(u_core, W0))
    return in_maps


def numpy_model(u_core, W0):
    """f32 numpy model of the routing (for small-scale checks)."""
    u_hat = np.einsum('pcoi,bpi->bpco', W0, u_core)
    b = np.zeros(u_hat.shape[:3], dtype=np.float32)
    v = None
    for _ in range(ROUTING_ITERS):
        e = np.exp(b - b.max(axis=2, keepdims=True))
        c = e / e.sum(axis=2, keepdims=True)
        s = np.einsum('bpc,bpco->bco', c, u_hat)
        sq = (s * s).sum(-1, keepdims=True)
        v = (sq / (1 + sq)) * s / np.sqrt(sq + EPS)
        b = b + np.einsum('bpco,bco->bpc', u_hat, v)
    return v


_COMPILED = {}


def _get_compiled():
    if "nc" in _COMPILED:
        return _COMPILED["nc"]
    from contextlib import ExitStack
    import concourse.tile as tile
    from concourse import bacc

    nc = bacc.Bacc("TRN2", target_bir_lowering=False, debug=False,
                   num_devices=NCORES)
    with tile.TileContext(nc) as tc:
        with ExitStack() as ctx:
            build(nc, tc, ctx)
    nc.compile()
    _COMPILED["nc"] = nc
    return nc


def kernel(u, W):
    """Full-input entry point: u [256,2048,8] f32, W [1,2048,10,16,8] f32
    -> v [256, 10, 16] f32."""
    from concourse.bass_utils import run_bass_kernel_spmd

    nc = _get_compiled()
    in_maps = make_inputs_per_core(u, W)
    res = run_bass_kernel_spmd(nc, in_maps, core_ids=list(range(NCORES)))
    outs = [res.results[c]["v_out"] for c in range(NCORES)]
    v = np.concatenate(outs, axis=0).reshape(B, C, O).astype(np.float32)
    return v
